# revision 29
# baseline (speedup 1.0000x reference)
"""Distributed GQA attention kernel for 8 TRN2 NeuronCores.

Sharding: core c = 4*b + k handles batch b (of 2) and kv-head k (of 4),
i.e. Q heads 4k..4k+3 (column-parallel qkv).  Attention is computed per
core in transposed layout (S^T = K Q^T per 128-key tile, causal-skipped),
then each head's attention output chunk ([128 hd x 512 t], bf16) is
AllGather-ed across the 4 cores of its batch group as soon as it is
ready, and each core computes a 512-column slice of the output
projection (column-parallel o_proj).  Host-side work is only sharding /
transpose / dtype cast / concat.

All matmuls run in bf16 (fp32 PSUM accumulation).  Softmax row sums are
built as two bf16 partial-sum tiles (alternating DVE / GPSIMD adds so
neither engine chains too long), then reduced across partitions by two
ones-vector matmuls into fp32 PSUM.
"""
import sys
import numpy as np

for _p in ("/root/.axon_site", "/root/.axon_site/_ro/trn_rl_repo",
           "/root/.axon_site/_ro/pypackages"):
    if _p not in sys.path:
        sys.path.append(_p)

import ml_dtypes  # noqa: E402
import concourse.bass as bass  # noqa: E402
from concourse import bacc  # noqa: E402
import concourse.mybir as mybir  # noqa: E402
from concourse import tile  # noqa: E402
import concourse.bass_utils as bass_utils  # noqa: E402

F32 = mybir.dt.float32
BF16 = mybir.dt.bfloat16
AL = mybir.AluOpType
ACTF = mybir.ActivationFunctionType
BF16NP = ml_dtypes.bfloat16

B, T, D = 2, 2048, 2048
H, HK, HD = 16, 4, 128
HPC = 4                      # q-heads per core
QCOLS = HPC * HD             # 512 q columns per core
CHUNK = 512                  # t-chunk
NE = D // 128                # contraction e-chunks
THETA = 10000.0
SCALE = 1.0 / float(np.sqrt(HD))
N_CORES = 8


def _consts(t=T):
    freqs = 1.0 / THETA ** (np.arange(0, HD, 2, dtype=np.float64) / HD)
    pos = np.arange(t, dtype=np.float64)
    ang = np.outer(freqs, pos)                            # [64, t]
    cos = np.cos(ang).astype(np.float32)
    sin = np.sin(ang).astype(np.float32)
    cos_full = np.concatenate([cos, cos], axis=0)         # [128, t]
    sin_pm = np.concatenate([-sin, sin], axis=0)          # [128, t]
    swap = np.zeros((128, 128), np.float32)
    swap[(np.arange(128) + 64) % 128, np.arange(128)] = 1.0
    ident = np.eye(128, dtype=np.float32)
    ones = np.ones((128, 128), np.float32)
    # triangular causal mask for the 128-col strip where a diagonal key
    # tile crosses the query range: tri[pp, jj] = 1 iff pp <= jj
    tri = (np.arange(128)[:, None] <= np.arange(128)[None, :]).astype(
        np.float32)
    return (cos_full, sin_pm, swap.astype(BF16NP), ident.astype(BF16NP),
            ones.astype(BF16NP), tri.astype(BF16NP))


def build(t=T):
    nchunk = t // CHUNK
    npt = t // 128
    nc = bacc.Bacc("TRN2", target_bir_lowering=False, debug=False,
                   num_devices=N_CORES)
    xT_e = nc.declare_dram_parameter("xT", [D, t], BF16, isOutput=False)
    wq_e = nc.declare_dram_parameter("wq", [D, QCOLS], BF16, isOutput=False)
    wk_e = nc.declare_dram_parameter("wk", [D, HD], BF16, isOutput=False)
    wv_e = nc.declare_dram_parameter("wv", [D, HD], BF16, isOutput=False)
    wo_e = nc.declare_dram_parameter("wo", [D, QCOLS], BF16, isOutput=False)
    out_e = nc.declare_dram_parameter("out", [t, QCOLS], F32, isOutput=True)

    cos_np, sinpm_np, swap_np, ident_np, ones_np, tri_np = _consts(t)
    cos_d = nc.inline_tensor(cos_np, "cos_c")
    sinpm_d = nc.inline_tensor(sinpm_np, "sinpm_c")
    swap_d = nc.inline_tensor(swap_np, "swap_c")
    ident_d = nc.inline_tensor(ident_np, "ident_c")
    ones_d = nc.inline_tensor(ones_np, "ones_c")
    tri_d = nc.inline_tensor(tri_np, "tri_c")

    groups = [[0, 1, 2, 3], [4, 5, 6, 7]]

    with tile.TileContext(nc) as tc:
        with (
            tc.tile_pool(name="wpool", bufs=1) as wpool,
            tc.tile_pool(name="xpool", bufs=32) as xpool,
            tc.tile_pool(name="kvpool", bufs=1) as kvpool,
            tc.tile_pool(name="work", bufs=2) as work,
            tc.tile_pool(name="ptpool", bufs=8) as ptpool,
            tc.tile_pool(name="aopool", bufs=8) as aopool,
            tc.tile_pool(name="oppool", bufs=34) as oppool,
            tc.tile_pool(name="qkv_ps", bufs=1, space="PSUM") as qkv_ps,
            tc.tile_pool(name="sc_ps", bufs=3, space="PSUM") as sc_ps,
            tc.tile_pool(name="av_ps", bufs=1, space="PSUM") as av_ps,
            tc.tile_pool(name="sm_ps", bufs=1, space="PSUM") as sm_ps,
            tc.tile_pool(name="op_ps", bufs=1, space="PSUM") as op_ps,
            tc.tile_pool(name="dram", bufs=1, space="DRAM") as dram,
        ):
            # ---- chunk-0 x tiles + projection weights first: they gate the
            # first matmul.  wo / attention consts are needed much later.
            # DMA emission order tracks the prologue's consumption order:
            # the K projection chain (x tile + small wk per e-step) can
            # start computing ~1.5us in and paces with DMA arrival.
            xts0 = []
            wq_sb, wk_sb, wv_sb, wo_sb = [], [], [], []
            for e in range(NE):
                xt = xpool.tile([128, CHUNK], BF16, name=f"xt0_{e}", tag="xt")
                nc.sync.dma_start(xt[:], xT_e[128 * e:128 * (e + 1), 0:CHUNK])
                xts0.append(xt)
                wk_t = wpool.tile([128, HD], BF16, name=f"wk{e}")
                nc.sync.dma_start(wk_t[:], wk_e[128 * e:128 * (e + 1), :])
                wk_sb.append(wk_t)
            for e in range(NE):
                wv_t = wpool.tile([128, HD], BF16, name=f"wv{e}")
                nc.sync.dma_start(wv_t[:], wv_e[128 * e:128 * (e + 1), :])
                wv_sb.append(wv_t)
            for e in range(NE):
                wq_t = wpool.tile([128, QCOLS], BF16, name=f"wq{e}")
                nc.sync.dma_start(wq_t[:], wq_e[128 * e:128 * (e + 1), :])
                wq_sb.append(wq_t)

            cos_sb = wpool.tile([128, t], F32, name="cos_sb")
            nc.sync.dma_start(cos_sb[:], cos_d[:, :])
            sinpm_sb = wpool.tile([128, t], F32, name="sinpm_sb")
            nc.sync.dma_start(sinpm_sb[:], sinpm_d[:, :])
            swap_sb = wpool.tile([128, 128], BF16, name="swap_sb")
            nc.sync.dma_start(swap_sb[:], swap_d[:, :])
            ident_sb = wpool.tile([128, 128], BF16, name="ident_sb")
            nc.sync.dma_start(ident_sb[:], ident_d[:, :])
            ones_sb = wpool.tile([128, 128], BF16, name="ones_sb")
            nc.sync.dma_start(ones_sb[:], ones_d[:, :])
            tri_sb = wpool.tile([128, 128], BF16, name="tri_sb")
            nc.sync.dma_start(tri_sb[:], tri_d[:, :])

            kT_sb = kvpool.tile([128, t], BF16, name="kT_sb")
            v_tiles = [kvpool.tile([128, HD], BF16, name=f"v{i}")
                       for i in range(npt)]

            def rope_pre(ps):
                """PSUM -> bf16 SBUF copy + half-swap matmul."""
                qsb = work.tile([128, CHUNK], BF16, tag="ropea", bufs=3)
                nc.vector.tensor_copy(qsb[:], ps[:])
                qsw = sm_ps.tile([128, CHUNK], F32, tag="sm")
                nc.tensor.matmul(qsw[:], swap_sb[:], qsb[:],
                                 start=True, stop=True)
                return qsb, qsw

            def rope_fin(qsb, qsw, out_ap, cols):
                t1 = work.tile([128, CHUNK], F32, tag="ropeb")
                nc.vector.tensor_tensor(t1[:], qsb[:], cos_sb[:, cols], AL.mult)
                t2 = work.tile([128, CHUNK], F32, tag="ropec")
                nc.vector.tensor_tensor(t2[:], qsw[:], sinpm_sb[:, cols],
                                        AL.mult)
                nc.vector.tensor_tensor(out_ap, t1[:], t2[:], AL.add)

            def emit_oproj_dmas(tcx, ccos):
                if not wo_sb:
                    for e in range(NE):
                        wo_t = wpool.tile([128, QCOLS], BF16, name=f"wo{e}")
                        nc.sync.dma_start(wo_t[:],
                                          wo_e[128 * e:128 * (e + 1), :])
                        wo_sb.append(wo_t)
                # gathered block (h, k) holds head 4k+h rows
                ats = {}
                for h in range(HPC):
                    for k in range(4):
                        at = oppool.tile([128, CHUNK], BF16,
                                         name=f"at{tcx}_{h}_{k}", tag="at")
                        nc.sync.dma_start(
                            at[:], ccos[h][128 * k:128 * (k + 1), :])
                        ats[(h, k)] = at
                return ats

            def emit_oproj_col(tcx, ats, j):
                # one 128-row output column block; accumulation is h-major
                # so early AllGathers are consumed first (matters only for
                # the final chunk, whose o_proj chases in-flight gathers)
                ops = op_ps.tile([128, QCOLS], F32, tag="op")
                n = 0
                for h in range(HPC):
                    for k in range(4):
                        nc.tensor.matmul(
                            ops[:], ats[(h, k)][:, 128 * j:128 * (j + 1)],
                            wo_sb[4 * k + h][:], start=(n == 0),
                            stop=(n == 15))
                        n += 1
                osb = work.tile([128, QCOLS], F32, tag="osb")
                nc.vector.tensor_copy(osb[:], ops[:])
                nc.sync.dma_start(
                    out_e[tcx * CHUNK + 128 * j:
                          tcx * CHUNK + 128 * (j + 1), :], osb[:])

            def emit_chain(cidx, kind, h, xts_c, qT_list):
                """one projection accumulation chain + its rope/transpose"""
                ccols = slice(cidx * CHUNK, (cidx + 1) * CHUNK)
                ps = qkv_ps.tile([128, CHUNK], F32, tag="qkv",
                                 name=f"ps_{cidx}_{kind}{h}")
                w_list = {"q": wq_sb, "k": wk_sb, "v": wv_sb}[kind]
                for e in range(NE):
                    w_ap = (w_list[e][:, 128 * h:128 * (h + 1)]
                            if kind == "q" else w_list[e][:])
                    nc.tensor.matmul(ps[:], w_ap, xts_c[e][:],
                                     start=(e == 0), stop=(e == NE - 1))
                if kind == "v":
                    vsb = work.tile([128, CHUNK], BF16, tag="vsb")
                    nc.vector.tensor_copy(vsb[:], ps[:])
                    for j in range(4):
                        tp = sm_ps.tile([128, 128], BF16, tag="sm")
                        nc.tensor.transpose(
                            tp[:], vsb[:, 128 * j:128 * (j + 1)],
                            ident_sb[:])
                        nc.vector.tensor_copy(
                            v_tiles[4 * cidx + j][:], tp[:])
                else:
                    qsb, qsw = rope_pre(ps)
                    if kind == "q":
                        qT = work.tile([128, CHUNK], BF16, tag="qT",
                                       bufs=10, name=f"qT{cidx}_{h}")
                        qT_list[h] = qT
                        rope_fin(qsb, qsw, qT[:], ccols)
                    else:
                        rope_fin(qsb, qsw, kT_sb[:, ccols], ccols)

            CHAIN_ORDER = [("k", 0), ("v", 0)] + [("q", h) for h in range(HPC)]
            pending_oproj = []
            xts_next = xts0
            qT_next = [None] * HPC
            # prologue: chunk-0 projections
            for kind, h in CHAIN_ORDER:
                emit_chain(0, kind, h, xts0, qT_next)

            for tcx in range(nchunk):
                xts = xts_next
                qT_heads = qT_next
                # filler work: PE instructions with no dependence on this
                # chunk's softmax pipeline, spread between attention heads
                fillers = []
                if tcx + 1 < nchunk:
                    ncols = slice((tcx + 1) * CHUNK, (tcx + 2) * CHUNK)
                    xts_next = []
                    for e in range(NE):
                        xt = xpool.tile([128, CHUNK], BF16,
                                        name=f"xt{tcx + 1}_{e}", tag="xt")
                        nc.sync.dma_start(
                            xt[:], xT_e[128 * e:128 * (e + 1), ncols])
                        xts_next.append(xt)
                    qT_next = [None] * HPC
                    qn = qT_next

                    def mk_chain(kind, ch, xc=xts_next, qq=qn, ci=tcx + 1):
                        return lambda: emit_chain(ci, kind, ch, xc, qq)

                    fillers = [mk_chain(kind, ch) for kind, ch in CHAIN_ORDER]
                while pending_oproj:
                    optcx, opccos = pending_oproj.pop(0)
                    ats_prev = emit_oproj_dmas(optcx, opccos)

                    def mk_col(j, ti=optcx, aa=ats_prev):
                        return lambda: emit_oproj_col(ti, aa, j)

                    fillers += [mk_col(j) for j in range(4)]

                # ---- attention for q-chunk tcx ----
                ccos = []
                n_pt = 4 * tcx + 4
                pending_a = None
                pending_b = None

                def tail_a(h, avp, sumA):
                    """partition-reduce the softmax partial sums + recip"""
                    rs = av_ps.tile([1, CHUNK], F32, tag="rs",
                                    name=f"rs{tcx}_{h}")
                    nc.tensor.matmul(rs[:], ones_sb[:, 0:1], sumA[:],
                                     start=True, stop=True)
                    recf = work.tile([1, CHUNK], F32, tag="recf",
                                     name=f"recf{tcx}_{h}")
                    nc.vector.reciprocal_approx_fast(recf[:], rs[:])
                    recip = work.tile([1, CHUNK], BF16, tag="recip",
                                      name=f"recip{tcx}_{h}")
                    nc.vector.tensor_copy(recip[:], recf[:])
                    return h, avp, recip

                def tail_b(h, avp, recip):
                    """broadcast recip, normalize, ship to the AllGather"""
                    bc = sm_ps.tile([128, CHUNK], F32, tag="sm")
                    nc.tensor.matmul(bc[:], ones_sb[0:1, :], recip[:],
                                     start=True, stop=True)
                    bc_sb = work.tile([128, CHUNK], F32, tag="bcsb")
                    nc.vector.tensor_copy(bc_sb[:], bc[:])
                    ao = aopool.tile([128, CHUNK], BF16, tag="ao")
                    nc.vector.tensor_tensor(ao[:], avp[:], bc_sb[:], AL.mult)
                    cci = dram.tile([128, CHUNK], BF16,
                                    name=f"cci{tcx}_{h}", tag=f"cci{tcx}_{h}")
                    nc.sync.dma_start(cci[:], ao[:])
                    cco = dram.tile([QCOLS, CHUNK], BF16,
                                    name=f"cco{tcx}_{h}", tag=f"cco{tcx}_{h}")
                    nc.gpsimd.collective_compute(
                        "AllGather", AL.bypass, replica_groups=groups,
                        ins=[cci.opt()], outs=[cco.opt()])
                    ccos.append(cco)

                for h in range(HPC):
                    avp = av_ps.tile([128, CHUNK], F32, tag="av",
                                     name=f"av{tcx}_{h}")
                    sumA = work.tile([128, CHUNK], BF16, tag="psumA",
                                     name=f"psA{tcx}_{h}")
                    pts = [None] * n_pt
                    # diagonal key tile p (= 4*tcx + i) only reaches query
                    # columns >= 128*i: slice every op to the live columns
                    lo = [max(0, 128 * (p - 4 * tcx)) for p in range(n_pt)]
                    for p in range(n_pt):
                        sps = sc_ps.tile([128, CHUNK], F32, tag="sc")
                        nc.tensor.matmul(
                            sps[:, lo[p]:], kT_sb[:, 128 * p:128 * (p + 1)],
                            qT_heads[h][:, lo[p]:], start=True, stop=True)
                        # previous head's softmax tail is staged into this
                        # head's score stream so the PE never waits on the
                        # ACT/DVE round-trips
                        if p == 1 and pending_a is not None:
                            pending_b = tail_a(*pending_a)
                            pending_a = None
                        elif p == 2 and pending_b is not None:
                            tail_b(*pending_b)
                            pending_b = None
                        pt = ptpool.tile([128, CHUNK], BF16, tag="pt")
                        nc.scalar.activation(pt[:, lo[p]:], sps[:, lo[p]:],
                                             ACTF.Exp, scale=SCALE)
                        if p >= 4 * tcx:
                            i = p - 4 * tcx
                            nc.vector.tensor_tensor(
                                pt[:, 128 * i:128 * (i + 1)],
                                pt[:, 128 * i:128 * (i + 1)],
                                tri_sb[:], AL.mult)
                        if p == 0:
                            nc.vector.tensor_copy(sumA[:], pt[:])
                        else:
                            nc.vector.tensor_tensor(
                                sumA[:, lo[p]:], sumA[:, lo[p]:],
                                pt[:, lo[p]:], AL.add)
                        pts[p] = pt
                        if p > 1:
                            nc.tensor.matmul(
                                avp[:, lo[p - 2]:], v_tiles[p - 2][:],
                                pts[p - 2][:, lo[p - 2]:],
                                start=(p == 2), stop=False)
                    for pp in (n_pt - 2, n_pt - 1):
                        nc.tensor.matmul(
                            avp[:, lo[pp]:], v_tiles[pp][:],
                            pts[pp][:, lo[pp]:],
                            start=(pp == 0), stop=(pp == n_pt - 1))
                    pending_a = (h, avp, sumA)
                    # dependency-free filler keeps the PE busy while the
                    # ACT/DVE softmax pipeline of this head drains
                    for _ in range(2):
                        if fillers:
                            fillers.pop(0)()
                if pending_a is not None:
                    pending_b = tail_a(*pending_a)
                    pending_a = None
                if pending_b is not None:
                    tail_b(*pending_b)
                    pending_b = None
                for f in fillers:
                    f()
                pending_oproj.append((tcx, ccos))

            # remaining o_projs; the final chunk's chases its own AllGathers
            for optcx, opccos in pending_oproj:
                ats_last = emit_oproj_dmas(optcx, opccos)
                for j in range(4):
                    emit_oproj_col(optcx, ats_last, j)
    nc.finalize()
    return nc


_NC_CACHE = None


def _get_nc():
    global _NC_CACHE
    if _NC_CACHE is None:
        _NC_CACHE = build()
    return _NC_CACHE


_HALF_PERM = np.concatenate([np.arange(0, HD, 2), np.arange(1, HD, 2)])


def _shard_inputs(x, wq, wk, wv, wo):
    perm_q = np.concatenate([128 * h + _HALF_PERM for h in range(HPC)])
    in_maps = []
    for c in range(N_CORES):
        b, k = c // 4, c % 4
        xT = np.ascontiguousarray(x[b].T.astype(BF16NP))
        wq_c = np.ascontiguousarray(
            wq[:, QCOLS * k:QCOLS * (k + 1)][:, perm_q].astype(BF16NP))
        wk_c = np.ascontiguousarray(
            wk[:, HD * k:HD * (k + 1)][:, _HALF_PERM].astype(BF16NP))
        wv_c = np.ascontiguousarray(
            wv[:, HD * k:HD * (k + 1)].astype(BF16NP))
        wo_c = np.ascontiguousarray(
            wo[:, QCOLS * k:QCOLS * (k + 1)].astype(BF16NP))
        in_maps.append({"xT": xT, "wq": wq_c, "wk": wk_c, "wv": wv_c,
                        "wo": wo_c})
    return in_maps


def kernel(x, wq, wk, wv, wo, _trace=False, _trace_kwargs=None):
    nc = _get_nc()
    in_maps = _shard_inputs(x, wq, wk, wv, wo)
    kw = {}
    if _trace:
        kw = dict(trace=True, **(_trace_kwargs or {}))
    res = bass_utils.run_bass_kernel_spmd(
        nc, in_maps, list(range(N_CORES)), **kw)
    t = x.shape[1]
    out = np.empty((B, t, D), np.float32)
    for c in range(N_CORES):
        b, k = c // 4, c % 4
        out[b][:, QCOLS * k:QCOLS * (k + 1)] = res.results[c]["out"]
    kernel.last_result = res
    return out
